# revision 7
# baseline (speedup 1.0000x reference)
"""ContrastiveCenterLoss on 8 Trainium2 NeuronCores — v2.

Math: with dist[b,c] = ||f_b - c_c||^2,
  intra = sum_b dist[b, label_b] = sum f^2 + sum cg^2 - 2*sum f.cg
          (cg = centers gathered by label)
  total = C*sum||f||^2 + B*sum||c||^2 - 2*(sum_b f_b)@(sum_c c_c)
  inter = total - intra
  loss  = (1/2/B) * intra / (inter + 1e-6) / 0.1

v2 changes vs baseline:
  - one dma_gather per chunk (SWDGE Q7 gather, 994ns fixed cost each)
    instead of 16 indirect_dma_start calls (16.6us of Pool time).
  - feat + centers-slice shipped as one bf16 blob (halved bytes, one
    HWDGE transaction); gather stays 512B/row-equivalent cost.
  - column sums via per-block matmul with ones as the *moving* operand
    (out [128,1] psum accumulated across blocks) -> all outputs are
    [128, k]; a single small output DMA.
  - gather split (896, 640, 512): desc-gen (994ns + 0.34ns/row per
    SWDGE call) pipelines against the transfers, and per-chunk compute
    waves start as each chunk's DMA semaphore fires.
  - per-engine instruction order pinned with 1-column overlapping
    writes (WAW deps) into shared scratch tiles; the Tile list
    scheduler otherwise reorders by its own readiness model and
    head-of-line blocks the engine queues.
Host all-reduces the per-core partial sums in float64 and applies the
final scalar division.
"""

import numpy as np
import ml_dtypes

B, C, D = 16384, 4096, 128
LAMBDA_C = 1.0
NCORES = 8
BS = B // NCORES          # 2048 feat rows per core
NB = BS // 128            # 16 feat row-blocks of 128
CSL = C // NCORES         # 512 center rows per core (stats slice)
CB = CSL // 128           # 4 cslice row-blocks
GCH = (896, 640, 512)     # gather chunk sizes (sum = BS)
NOCOL = 12                # output columns
X3A = 432                 # square rows of chunk 3 on ACT (rest on DVE)

_cached = {}


def _build_nc():
    import concourse.bass as bass
    import concourse.tile as tile
    from concourse import bacc, mybir

    f32 = mybir.dt.float32
    bf16 = mybir.dt.bfloat16
    i16 = mybir.dt.int16
    mult = mybir.AluOpType.mult

    nc = bacc.Bacc("TRN2", target_bir_lowering=False, debug=False,
                   num_devices=NCORES, dynamic_dma_scratch_size=65536)

    idxt = nc.dram_tensor("idxt", [128, BS // 16], i16, kind="ExternalInput")
    blob = nc.dram_tensor("blob", [128, (NB + CB) * D], bf16,
                          kind="ExternalInput")
    censb = nc.dram_tensor("censb", [C, D], bf16, kind="ExternalInput")
    # cols: 0 f2a 1 f2b 2 csq 3 F 4 Cv | 5,6,7,11 cross | 8,9,10,12 cgsq
    o_all = nc.dram_tensor("o_all", [128, NOCOL], f32, kind="ExternalOutput")

    FW = NB * D               # 2048 feat free cols
    CW = CB * D               # 512 cslice free cols

    with tile.TileContext(nc) as tc:
        with tc.tile_pool(name="const", bufs=1) as cpool, \
             tc.tile_pool(name="sbuf", bufs=1) as pool, \
             tc.tile_pool(name="psum", bufs=1, space="PSUM") as psum:

            ones = cpool.tile([128, 1], bf16, tag="ones")
            nc.vector.memset(ones[:], 1.0)

            # index tile first so the gather chain starts ASAP (HWDGE)
            idx_t = pool.tile([128, BS // 16], i16, tag="idx")
            nc.sync.dma_start(out=idx_t[:], in_=idxt.ap())

            # feat + cslice blob via SWDGE so its desc-gen overlaps the
            # idx HWDGE transactions and the transfer starts early
            bl_t = pool.tile([128, (NB + CB) * D], bf16, tag="blob")
            nc.gpsimd.dma_start(out=bl_t[:], in_=blob.ap())
            f_v = bl_t[:, 0:FW]
            cs_v = bl_t[:, FW:FW + CW]

            cg_t = pool.tile([128, FW], bf16, tag="cg")
            cg3 = cg_t[:].rearrange("p (n d) -> p n d", d=D)

            o_t = pool.tile([128, NOCOL], f32, tag="o")

            # gathers: centers rows by label, chunked
            s = 0
            for gi, n in enumerate(GCH):
                nc.gpsimd.dma_gather(
                    cg3[:, s // 128:(s + n) // 128, :],
                    censb.ap(),
                    idx_t[:, s // 16:(s + n) // 16],
                    n, n, D,
                )
                s += n

            # Output columns:
            # 0 f2a(DVE) 1 f2b(ACT) 2 csq 3 F 4 Cv
            # 5,6,7 cross1..3 (DVE)  8 cg1^2 9 cg2^2 10 cg3a^2 (ACT)
            # 11 cg3b^2 (DVE)
            SQ = mybir.ActivationFunctionType.Square

            # DVE stream: f2a, psum copies, cross1..3, cg3b^2 — pinned
            # by overlapping spans in s_d
            c1, c2, c3 = GCH
            b1, b2 = c1, c1 + c2
            dsp = [0, 1023, 1023 + c1 - 1, 1023 + c1 + c2 - 2,
                   1023 + c1 + c2 + c3 - 3]
            s_d = pool.tile([128, dsp[4] + (c3 - X3A) + 8], bf16, tag="s_d")
            nc.vector.scalar_tensor_tensor(
                out=s_d[:, 0:1024], in0=f_v[:, 0:1024], scalar=1.0,
                in1=f_v[:, 0:1024], op0=mult, op1=mult,
                accum_out=o_t[:, 0:1])

            # ACT stream: cs^2, f2b, cg1^2, cg2^2, cg3a^2 — pinned in s_a
            asp = [0, 511, 511 + 1024 - 1, 511 + 1024 + c1 - 2,
                   511 + 1024 + c1 + c2 - 3]
            s_a = pool.tile([128, asp[4] + X3A + 8], bf16, tag="s_a")
            nc.scalar.activation(
                out=s_a[:, 0:CW], in_=cs_v, func=SQ, accum_out=o_t[:, 2:3])
            nc.scalar.activation(
                out=s_a[:, asp[1]:asp[1] + 1024], in_=f_v[:, 1024:2048],
                func=SQ, accum_out=o_t[:, 1:2])

            # column sums: per-block matmul, data stationary, ones moving
            psF = psum.tile([128, 1], f32, tag="psF")
            f3 = bl_t[:].rearrange("p (n d) -> p n d", d=D)
            for n in range(NB):
                nc.tensor.matmul(out=psF[:], lhsT=f3[:, n, :], rhs=ones[:],
                                 start=(n == 0), stop=(n == NB - 1))
            psC = psum.tile([128, 1], f32, tag="psC")
            for n in range(CB):
                nc.tensor.matmul(out=psC[:], lhsT=f3[:, NB + n, :],
                                 rhs=ones[:],
                                 start=(n == 0), stop=(n == CB - 1))
            nc.vector.tensor_copy(o_t[:, 3:4], psF[:])
            nc.vector.tensor_copy(o_t[:, 4:5], psC[:])

            # gather-gated: crosses on DVE, squares on ACT; the wave-3
            # square is split ACT/DVE (X3A rows on ACT)
            nc.vector.scalar_tensor_tensor(
                out=s_d[:, dsp[1]:dsp[1] + c1], in0=f_v[:, 0:b1],
                scalar=1.0, in1=cg_t[:, 0:b1], op0=mult, op1=mult,
                accum_out=o_t[:, 5:6])
            nc.vector.scalar_tensor_tensor(
                out=s_d[:, dsp[2]:dsp[2] + c2], in0=f_v[:, b1:b2],
                scalar=1.0, in1=cg_t[:, b1:b2], op0=mult, op1=mult,
                accum_out=o_t[:, 6:7])
            nc.vector.scalar_tensor_tensor(
                out=s_d[:, dsp[3]:dsp[3] + c3], in0=f_v[:, b2:FW],
                scalar=1.0, in1=cg_t[:, b2:FW], op0=mult, op1=mult,
                accum_out=o_t[:, 7:8])
            nc.vector.scalar_tensor_tensor(
                out=s_d[:, dsp[4]:dsp[4] + (c3 - X3A)],
                in0=cg_t[:, b2 + X3A:FW], scalar=1.0,
                in1=cg_t[:, b2 + X3A:FW], op0=mult, op1=mult,
                accum_out=o_t[:, 11:12])
            nc.scalar.activation(
                out=s_a[:, asp[2]:asp[2] + c1], in_=cg_t[:, 0:b1], func=SQ,
                accum_out=o_t[:, 8:9])
            nc.scalar.activation(
                out=s_a[:, asp[3]:asp[3] + c2], in_=cg_t[:, b1:b2],
                func=SQ, accum_out=o_t[:, 9:10])
            nc.scalar.activation(
                out=s_a[:, asp[4]:asp[4] + X3A], in_=cg_t[:, b2:b2 + X3A],
                func=SQ, accum_out=o_t[:, 10:11])

            nc.sync.dma_start(out=o_all.ap(), in_=o_t[:])

    nc.compile()
    return nc


def _get_nc():
    if "nc" not in _cached:
        _cached["nc"] = _build_nc()
    return _cached["nc"]


def _make_in_maps(feat, label, centers):
    feat = np.asarray(feat, dtype=np.float32)
    centers = np.asarray(centers, dtype=np.float32)
    lab = np.asarray(label).astype(np.int16)

    bf = ml_dtypes.bfloat16
    censb = np.ascontiguousarray(centers.astype(bf))

    in_maps = []
    for k in range(NCORES):
        fs = feat[k * BS:(k + 1) * BS].astype(bf)
        # row i -> partition i%128, block i//128
        fs = fs.reshape(NB, 128, D).transpose(1, 0, 2)
        cs = centers[k * CSL:(k + 1) * CSL].astype(bf)
        cs = cs.reshape(CB, 128, D).transpose(1, 0, 2)
        blob = np.ascontiguousarray(
            np.concatenate([fs.reshape(128, NB * D),
                            cs.reshape(128, CB * D)], axis=1))
        # gather idx layout: position i read from idx[i%16, i//16];
        # partitions 16..127 replicate (must hold valid indices)
        ls = lab[k * BS:(k + 1) * BS]
        m16 = np.ascontiguousarray(ls.reshape(BS // 16, 16).T)  # [16, BS/16]
        idx = np.ascontiguousarray(np.tile(m16, (8, 1)))        # [128, BS/16]
        in_maps.append({
            "idxt": idx,
            "blob": blob,
            "censb": censb,
        })
    return in_maps


def _combine(results):
    fsq = 0.0
    cross = 0.0
    cgsq = 0.0
    csq = 0.0
    F = np.zeros(D, dtype=np.float64)
    Cv = np.zeros(D, dtype=np.float64)
    for r in results:
        a = r["o_all"].astype(np.float64)
        fsq += a[:, 0:2].sum()
        csq += a[:, 2].sum()
        F += a[:, 3]
        Cv += a[:, 4]
        cross += a[:, 5:8].sum()
        cgsq += a[:, 8:12].sum()
    intra = fsq + cgsq - 2.0 * cross
    total = C * fsq + B * csq - 2.0 * float(F @ Cv)
    inter = total - intra
    loss = (LAMBDA_C / 2.0 / B) * intra / (inter + 1e-6) / 0.1
    return np.float32(loss)


def kernel(feat, label, centers):
    from concourse.bass_utils import run_bass_kernel_spmd

    nc = _get_nc()
    in_maps = _make_in_maps(feat, label, centers)
    res = run_bass_kernel_spmd(nc, in_maps, list(range(NCORES)))
    return _combine(res.results)
